# revision 1
# baseline (speedup 1.0000x reference)
"""Trainium2 Bass kernel for nn_BinaryLinear: out = sign(x @ sign(W).T + bias).

Strategy
--------
Data-parallel over the 8192-token dim: each of the 8 cores gets 1024 tokens
and the full weight matrix.

On-chip compute (per core) is the NT GEMM z.T = sign(W) @ x.T on the
TensorEngine with the contraction (in_features) on the partition dim:

  psum[outf, tok] = sum_k w_b_T[k, outf] * x_T[k, tok]

Both operands are pre-transposed on the host (pure layout prep) so every DMA
is contiguous-per-partition. Precision/speed: x is split as

  x ~= fp16(x) + 2^-6 * e4m3((x - fp16(x)) * 2^6)        (~15-16 mantissa bits)

The hi half runs as regular fp16 matmuls (1 PE cycle/row). The lo half runs
as fp8e4m3 DoubleRow matmuls (2x FLOPs per instruction, 256-deep contraction)
with the 2^-6 scale folded into the fp8 weights (+-2^-6 is exact in e4m3),
so BOTH halves accumulate into the same fp32 PSUM group with no epilogue
combine. Combined error lands at the fp32 reference's own accumulation-error
scale. fp32 matmul would be 4 cycles/row; a bf16 hi+lo split is 2 cycles/row;
this scheme is ~1.5.

sign(W) is computed on-chip (ScalarE Sign: fp32 -> fp16 +-1, then VectorE
*2^-6 -> e4m3). The epilogue fuses bias-add + sign + PSUM->SBUF in one
ScalarE activation (bias is per-partition in the z.T layout). Output is
written as z.T [out_features, tokens] per core and untransposed on the host.

Within each PSUM group all fp16 MMs run before the DoubleRow MMs, so the lo
x data is needed one hi-phase later than the hi data and the single serial
gpsimd DMA stream (hi chunks in k order, then lo chunks) stays ahead of the
PE after the first iteration. Measured on HW: ~710us per core (PE matmul
roofline for this scheme is ~654us).
"""

import numpy as np

import concourse.tile as tile
import concourse.mybir as mybir
from concourse import bacc
from concourse.bass_utils import run_bass_kernel_spmd
from concourse.tile_rust import add_dep_helper

N_CORES = 8
N_TOK = 8192
D_IN = 4096
D_OUT = 4096
P = 128
T = N_TOK // N_CORES  # 1024 tokens per core
KT = D_IN // P  # 32 contraction tiles
KP = KT // 2  # 16 DoubleRow k-pairs
MT = D_OUT // P  # 32 out-feature tiles
M2 = 2  # m-tiles per cached W block (256 outf cols)
MB = MT // M2  # 16 W blocks
TB = 512  # token block (one PSUM bank of fp32)
NB = T // TB  # 2 token blocks per core
LO_SCALE = 2.0 ** 6  # host-side scale on the fp8 residual; inverse on weights

F32 = mybir.dt.float32
FP16 = mybir.dt.float16
FP8 = mybir.dt.float8e4
SIGN = mybir.ActivationFunctionType.Sign
DR = mybir.MatmulPerfMode.DoubleRow
E4M3 = mybir.dt.np(FP8)

_nc_cache = None


def build():
    """Build + compile the per-core Bass/Tile module (SPMD: same on all cores)."""
    global _nc_cache
    if _nc_cache is not None:
        return _nc_cache
    nc = bacc.Bacc("TRN2", target_bir_lowering=False, debug=False, num_devices=N_CORES)
    xhi_d = nc.dram_tensor("x_hi_t", [D_IN, T], FP16, kind="ExternalInput").ap()
    xlo_d = nc.dram_tensor("x_lo8_t", [D_IN, T], FP8, kind="ExternalInput").ap()
    w_d = nc.dram_tensor("w_t", [D_IN, D_OUT], F32, kind="ExternalInput").ap()
    b_d = nc.dram_tensor("bias", [D_OUT], F32, kind="ExternalInput").ap()
    out_d = nc.dram_tensor("out_t", [D_OUT, T], F32, kind="ExternalOutput").ap()

    with tile.TileContext(nc) as tc:
        with (
            tc.tile_pool(name="x", bufs=1) as x_pool,
            tc.tile_pool(name="wstage", bufs=8) as wstage_pool,
            tc.tile_pool(name="wsb", bufs=3) as w_pool,
            tc.tile_pool(name="bias", bufs=1) as b_pool,
            tc.tile_pool(name="out", bufs=6) as out_pool,
            tc.tile_pool(name="psum", bufs=8, space="PSUM") as psum_pool,
        ):
            def convert_w_block(mb):
                # Stage a [D_IN, 256] W column block; convert to
                # sign() in fp16 (+-1) and e4m3 (+-2^-6).
                wsb_hi = w_pool.tile([P, KT, M2 * P], FP16, tag="wsb_hi",
                                     name=f"wsb_hi_{mb}")
                wsb_lo = w_pool.tile([P, KT, M2 * P], FP8, tag="wsb_lo",
                                     name=f"wsb_lo_{mb}")
                for k in range(KT):
                    wstage = wstage_pool.tile([P, M2 * P], F32, tag="wstage",
                                              name=f"wstage_{mb}_{k}")
                    nc.sync.dma_start(
                        wstage[:],
                        w_d[k * P : (k + 1) * P, mb * M2 * P : (mb + 1) * M2 * P],
                    )
                    nc.scalar.activation(wsb_hi[:, k, :], wstage[:], SIGN)
                    nc.vector.tensor_scalar_mul(
                        wsb_lo[:, k, :], wsb_hi[:, k, :], 1.0 / LO_SCALE
                    )
                return wsb_hi, wsb_lo

            # mb0's W conversion is emitted first so its ScalarE/VectorE ops
            # are not queued behind anything on those engines.
            wsb_cache = {0: convert_w_block(0)}

            # Resident x, chunked per k-tile (full token width) so matmuls
            # depend on exactly the chunk they read, all on the gpsimd queue
            # (the sync queue streams W).
            # The tail half of the hi chunks and all lo chunks are gated on
            # early mb0 compute (add_dep_helper below) so the chunks the PE
            # needs first get the full DMA-ring bandwidth instead of
            # fair-sharing it with everything in flight.
            xhi = []
            xlo8 = []
            hi_tail_dmas = []
            lo_dmas = []
            for ko in range(KT):
                th = x_pool.tile([P, T], FP16, tag=f"xh_{ko}", name=f"xh_{ko}")
                dma = nc.gpsimd.dma_start(th[:], xhi_d[ko * P : (ko + 1) * P, :])
                if ko >= 16:
                    hi_tail_dmas.append(dma.ins)
                xhi.append(th)
            for t2 in range(KP):
                tl = x_pool.tile([P, 2, T], FP8, tag=f"xl_{t2}", name=f"xl_{t2}")
                for j in range(2):
                    ko = 2 * t2 + j
                    dma = nc.gpsimd.dma_start(
                        tl[:, j, :], xlo_d[ko * P : (ko + 1) * P, :]
                    )
                    lo_dmas.append(dma.ins)
                xlo8.append(tl)
            gate_hi = gate_lo = None  # mb0 MMs at k=4 / k=12
            # bias, outf-partition-major: bias_sb[p, mo] = bias[mo*128 + p]
            bias_sb = b_pool.tile([P, MT], F32, tag="bias")
            nc.sync.dma_start(bias_sb[:], b_d.rearrange("(mo p) -> p mo", p=P))

            for mb in range(MB):
                if mb not in wsb_cache:
                    wsb_cache[mb] = convert_w_block(mb)
                wsb_hi, wsb_lo = wsb_cache.pop(mb)

                # Both token-blocks interleaved inside the k loop: each
                # weight load (LDWEIGHTS) feeds two 512-col matmuls, so the
                # weight-load stream is fully hidden. 4 PSUM groups live
                # (M2 x NB) = 4 banks; bufs=8 double-buffers across mb.
                nsls = [slice(n * TB, (n + 1) * TB) for n in range(NB)]
                psums = {
                    (mi, n): psum_pool.tile([P, TB], F32, tag="psum",
                                            name=f"ps_{mb}_{n}_{mi}")
                    for mi in range(M2)
                    for n in range(NB)
                }
                for k in range(KT):
                    for mi in range(M2):
                        msl = slice(mi * P, (mi + 1) * P)
                        for n in range(NB):
                            mm = nc.tensor.matmul(
                                psums[(mi, n)][:],
                                wsb_hi[:, k, msl],
                                xhi[k][:, nsls[n]],
                                start=(k == 0),
                                stop=False,
                            )
                            if mb == 0 and mi == M2 - 1 and n == NB - 1:
                                if k == 4:
                                    gate_hi = mm.ins
                                elif k == 12:
                                    gate_lo = mm.ins
                for t in range(KP):
                    for mi in range(M2):
                        msl = slice(mi * P, (mi + 1) * P)
                        for n in range(NB):
                            nc.tensor.matmul(
                                psums[(mi, n)][:],
                                wsb_lo[:, 2 * t : 2 * t + 2, msl],
                                xlo8[t][:, :, nsls[n]],
                                start=False,
                                stop=(t == KP - 1),
                                perf_mode=DR,
                            )
                for mi in range(M2):
                    m = mb * M2 + mi
                    for n in range(NB):
                        osb = out_pool.tile([P, TB], F32, tag="osb",
                                            name=f"osb_{mb}_{n}_{mi}")
                        nc.scalar.activation(
                            osb[:], psums[(mi, n)][:], SIGN,
                            bias=bias_sb[:, m : m + 1],
                        )
                        nc.sync.dma_start(
                            out_d[m * P : (m + 1) * P, nsls[n]], osb[:]
                        )
    nc.compile()
    _nc_cache = nc
    return nc


def prep_in_maps(x, weight, bias):
    """Host-side layout prep: fp16/fp8 split of x, transposes, token shards."""
    x = np.asarray(x, dtype=np.float32)
    weight = np.asarray(weight, dtype=np.float32)
    bias = np.asarray(bias, dtype=np.float32)

    x_hi = x.astype(np.float16)
    x_lo8 = ((x - x_hi.astype(np.float32)) * LO_SCALE).astype(E4M3)
    xhi_t = np.ascontiguousarray(x_hi.T)  # [D_IN, N_TOK]
    xlo_t = np.ascontiguousarray(x_lo8.T)
    w_t = np.ascontiguousarray(weight.T)  # [D_IN, D_OUT]

    in_maps = []
    for c in range(N_CORES):
        sl = slice(c * T, (c + 1) * T)
        in_maps.append(
            {
                "x_hi_t": np.ascontiguousarray(xhi_t[:, sl]),
                "x_lo8_t": np.ascontiguousarray(xlo_t[:, sl]),
                "w_t": w_t,
                "bias": bias,
            }
        )
    return in_maps


def run(x, weight, bias, **spmd_kwargs):
    """Run on the 8 cores; returns (full_output, BassKernelResults)."""
    nc = build()
    in_maps = prep_in_maps(x, weight, bias)
    res = run_bass_kernel_spmd(nc, in_maps, core_ids=list(range(N_CORES)), **spmd_kwargs)
    out = np.empty((N_TOK, D_OUT), dtype=np.float32)
    for c in range(N_CORES):
        out[c * T : (c + 1) * T, :] = res.results[c]["out_t"].T
    return out, res


def kernel(x, weight, bias):
    out, _ = run(x, weight, bias)
    return out



# revision 2
# speedup vs baseline: 1.7659x; 1.7659x over previous
"""Trainium2 Bass kernel for nn_BinaryLinear: out = sign(x @ sign(W).T + bias).

Strategy
--------
Data-parallel over the 8192-token dim: each of the 8 cores gets 1024 tokens
and the full weight matrix.

On-chip compute (per core) is the NT GEMM z.T = sign(W) @ x.T on the
TensorEngine with the contraction (in_features) on the partition dim:

  psum[outf, tok] = sum_k w_s[k, outf] * x16[k, tok]

Precision: x is rounded to fp16 on the host. Host-side simulation of the
exact quantization error (acc_sim.py) shows 2195/33.5M sign flips vs the
fp32 reference => rel_err 0.0162 < 2e-2 gate (the sim matched HW flip
counts exactly for the previous fp16+fp8 scheme: 97 predicted, 97
measured). PE accumulation is fp32 and adds nothing measurable.

sign(W) is computed on the host and shipped as fp8e4 (+-1 exact), already
packed in the SBUF layout the PE wants (per-partition-contiguous), so every
DMA is a full-line contiguous transfer and no on-chip conversion is needed.
The matmul runs with fp8 stationary x fp16 moving at 1 col/cycle.

The epilogue fuses bias-add + sign + PSUM->SBUF in one ScalarE activation
(bias is per-partition in the z.T layout), writing fp8 (+-1 exact), which
quarters the output DMA. Host converts back to fp32 and untransposes.

DMA queues: x + bias on sync (HWDGE), W blocks on gpsimd (SWDGE),
outputs on scalar (HWDGE) so the three streams don't serialize behind
each other in one FIFO.
"""

import numpy as np

import concourse.tile as tile
import concourse.mybir as mybir
from concourse import bacc
from concourse.bass_utils import run_bass_kernel_spmd

N_CORES = 8
N_TOK = 8192
D_IN = 4096
D_OUT = 4096
P = 128
T = N_TOK // N_CORES  # 1024 tokens per core
KT = D_IN // P  # 32 contraction tiles
MT = D_OUT // P  # 32 out-feature tiles
M2 = 2  # m-tiles per W block (256 outf cols)
MB = MT // M2  # 16 W blocks
TB = 512  # token block (one PSUM bank of fp32)
NB = T // TB  # 2 token blocks per core
XG = 4  # k-tiles per x DMA chunk
NXG = KT // XG  # 8 x DMA chunks

F32 = mybir.dt.float32
FP16 = mybir.dt.float16
FP8 = mybir.dt.float8e4
SIGN = mybir.ActivationFunctionType.Sign
E4M3 = mybir.dt.np(FP8)

_nc_cache = None


def build():
    """Build + compile the per-core Bass/Tile module (SPMD: same on all cores)."""
    global _nc_cache
    if _nc_cache is not None:
        return _nc_cache
    nc = bacc.Bacc("TRN2", target_bir_lowering=False, debug=False, num_devices=N_CORES)
    x_d = nc.dram_tensor("x16", [P, KT * T], FP16, kind="ExternalInput").ap()
    w_d = nc.dram_tensor("w8", [MB * P, KT * M2 * P], FP8, kind="ExternalInput").ap()
    b_d = nc.dram_tensor("bias_pk", [P, MT], F32, kind="ExternalInput").ap()
    out_d = nc.dram_tensor("out8_t", [D_OUT, T], FP8, kind="ExternalOutput").ap()

    with tile.TileContext(nc) as tc:
        with (
            tc.tile_pool(name="x", bufs=1) as x_pool,
            tc.tile_pool(name="wsb", bufs=3) as w_pool,
            tc.tile_pool(name="bias", bufs=1) as b_pool,
            tc.tile_pool(name="out", bufs=6) as out_pool,
            tc.tile_pool(name="psum", bufs=8, space="PSUM") as psum_pool,
        ):
            # bias, outf-partition-major: bias_sb[p, mo] = bias[mo*128 + p]
            bias_sb = b_pool.tile([P, MT], F32, tag="bias")
            nc.sync.dma_start(bias_sb[:], b_d[:, :])

            # Resident x, 8 chunks of 4 k-tiles (1 MB each) so the first
            # matmuls start after ~one chunk lands and the stream stays
            # ahead of the PE.
            xt = []
            for g in range(NXG):
                th = x_pool.tile([P, XG, T], FP16, tag=f"xh_{g}", name=f"xh_{g}")
                nc.sync.dma_start(th[:], x_d[:, g * XG * T : (g + 1) * XG * T])
                xt.append(th)

            def x_sl(k, n):
                return xt[k // XG][:, k % XG, n * TB : (n + 1) * TB]

            for mb in range(MB):
                wsb = w_pool.tile([P, KT, M2 * P], FP8, tag="wsb",
                                  name=f"wsb_{mb}")
                nc.gpsimd.dma_start(wsb[:], w_d[mb * P : (mb + 1) * P, :])

                psums = {
                    (mi, n): psum_pool.tile([P, TB], F32, tag="psum",
                                            name=f"ps_{mb}_{n}_{mi}")
                    for mi in range(M2)
                    for n in range(NB)
                }
                for k in range(KT):
                    for mi in range(M2):
                        msl = slice(mi * P, (mi + 1) * P)
                        for n in range(NB):
                            nc.tensor.matmul(
                                psums[(mi, n)][:],
                                wsb[:, k, msl],
                                x_sl(k, n),
                                start=(k == 0),
                                stop=(k == KT - 1),
                            )
                for mi in range(M2):
                    m = mb * M2 + mi
                    for n in range(NB):
                        osb = out_pool.tile([P, TB], FP8, tag="osb",
                                            name=f"osb_{mb}_{n}_{mi}")
                        nc.scalar.activation(
                            osb[:], psums[(mi, n)][:], SIGN,
                            bias=bias_sb[:, m : m + 1],
                        )
                        nc.scalar.dma_start(
                            out_d[m * P : (m + 1) * P, n * TB : (n + 1) * TB],
                            osb[:],
                        )
    nc.compile()
    _nc_cache = nc
    return nc


def prep_in_maps(x, weight, bias):
    """Host-side layout prep: fp16 cast of x, sign(W)->fp8, packing."""
    x = np.asarray(x, dtype=np.float32)
    weight = np.asarray(weight, dtype=np.float32)
    bias = np.asarray(bias, dtype=np.float32)

    x16 = x.astype(np.float16)
    # w8[p, k, mi, c] = sign(W)[mb*256 + mi*128 + c, k*128 + p], per block mb
    S = np.sign(weight).astype(np.float32)
    w8 = (
        S.reshape(MB, M2, P, KT, P)
        .transpose(0, 4, 3, 1, 2)  # [mb, p, k, mi, c]
        .reshape(MB * P, KT * M2 * P)
    ).astype(E4M3)
    w8 = np.ascontiguousarray(w8)
    bias_pk = np.ascontiguousarray(bias.reshape(MT, P).T)

    in_maps = []
    for c in range(N_CORES):
        xc = x16[c * T : (c + 1) * T]  # [T, D_IN]
        # xp[p, k, t] = x16[c*T + t, k*128 + p]
        xp = np.ascontiguousarray(
            xc.reshape(T, KT, P).transpose(2, 1, 0).reshape(P, KT * T)
        )
        in_maps.append({"x16": xp, "w8": w8, "bias_pk": bias_pk})
    return in_maps


def run(x, weight, bias, **spmd_kwargs):
    """Run on the 8 cores; returns (full_output, BassKernelResults)."""
    nc = build()
    in_maps = prep_in_maps(x, weight, bias)
    res = run_bass_kernel_spmd(nc, in_maps, core_ids=list(range(N_CORES)), **spmd_kwargs)
    out = np.empty((N_TOK, D_OUT), dtype=np.float32)
    for c in range(N_CORES):
        out[c * T : (c + 1) * T, :] = res.results[c]["out8_t"].astype(np.float32).T
    return out, res


def kernel(x, weight, bias):
    out, _ = run(x, weight, bias)
    return out


# revision 4
# speedup vs baseline: 1.7990x; 1.0187x over previous
"""Trainium2 Bass kernel for nn_BinaryLinear: out = sign(x @ sign(W).T + bias).

Strategy
--------
Data-parallel over the 8192-token dim: each of the 8 cores gets 1024 tokens
and the full weight matrix.

On-chip compute (per core) is the NT GEMM z.T = sign(W) @ x.T on the
TensorEngine with the contraction (in_features) on the partition dim:

  psum[outf, tok] = sum_k w_s[k, outf] * x16[k, tok]

Precision: x is rounded to fp16 on the host. Host-side simulation of the
exact quantization error (acc_sim.py) shows 2195/33.5M sign flips vs the
fp32 reference => rel_err 0.0162 < 2e-2 gate (the sim matched HW flip
counts exactly for the previous fp16+fp8 scheme: 97 predicted, 97
measured). PE accumulation is fp32 and adds nothing measurable.

sign(W) is computed on the host and shipped as fp8e4 (+-1 exact), already
packed in the SBUF layout the PE wants (per-partition-contiguous), so every
DMA is a full-line contiguous transfer and no on-chip conversion is needed.
The matmul runs with fp8 stationary x fp16 moving at 1 col/cycle.

The epilogue fuses bias-add + sign + PSUM->SBUF in one ScalarE activation
(bias is per-partition in the z.T layout), writing fp8 (+-1 exact), which
quarters the output DMA. Host converts back to fp32 and untransposes.

DMA queues: x + bias on sync (HWDGE), W blocks on gpsimd (SWDGE),
outputs on scalar (HWDGE) so the three streams don't serialize behind
each other in one FIFO.
"""

import numpy as np

import concourse.tile as tile
import concourse.mybir as mybir
from concourse import bacc
from concourse.bass_utils import run_bass_kernel_spmd
from concourse.tile_rust import add_dep_helper

N_CORES = 8
N_TOK = 8192
D_IN = 4096
D_OUT = 4096
P = 128
T = N_TOK // N_CORES  # 1024 tokens per core
KT = D_IN // P  # 32 contraction tiles
MT = D_OUT // P  # 32 out-feature tiles
M2 = 2  # m-tiles per W block (256 outf cols)
MB = MT // M2  # 16 W blocks
TB = 512  # token block (one PSUM bank of fp32)
NB = T // TB  # 2 token blocks per core
XG = 4  # k-tiles per x DMA chunk
NXG = KT // XG  # 8 x DMA chunks

F32 = mybir.dt.float32
FP16 = mybir.dt.float16
FP8 = mybir.dt.float8e4
SIGN = mybir.ActivationFunctionType.Sign
E4M3 = mybir.dt.np(FP8)

_nc_cache = None


def build():
    """Build + compile the per-core Bass/Tile module (SPMD: same on all cores)."""
    global _nc_cache
    if _nc_cache is not None:
        return _nc_cache
    nc = bacc.Bacc("TRN2", target_bir_lowering=False, debug=False, num_devices=N_CORES)
    x_d = nc.dram_tensor("x16", [P, KT * T], FP16, kind="ExternalInput").ap()
    w_d = nc.dram_tensor("w8", [MB * P, KT * M2 * P], FP8, kind="ExternalInput").ap()
    b_d = nc.dram_tensor("bias_pk", [P, MT], F32, kind="ExternalInput").ap()
    out_d = nc.dram_tensor("out8_t", [D_OUT, T], FP8, kind="ExternalOutput").ap()

    with tile.TileContext(nc) as tc:
        with (
            tc.tile_pool(name="x", bufs=1) as x_pool,
            tc.tile_pool(name="wsb", bufs=3) as w_pool,
            tc.tile_pool(name="bias", bufs=1) as b_pool,
            tc.tile_pool(name="out", bufs=6) as out_pool,
            tc.tile_pool(name="warm", bufs=1) as warm_pool,
            tc.tile_pool(name="psum", bufs=8, space="PSUM") as psum_pool,
        ):
            # PE warmup: the HAM clock gate holds the PE at 1.2 GHz until it
            # has been busy ~3.4us. Run dummy matmuls on a zeroed tile while
            # the first x/W DMAs are in flight so the real matmuls start at
            # 2.4 GHz.
            warm = warm_pool.tile([P, TB], FP16, tag="warm")
            nc.vector.memset(warm[:], 0.0)
            wps = psum_pool.tile([P, TB], F32, tag="psum", name="warm_ps")
            for i in range(24):
                nc.tensor.matmul(wps[:], warm[:, :P], warm[:], start=True,
                                 stop=True)

            # bias, outf-partition-major: bias_sb[p, mo] = bias[mo*128 + p]
            bias_sb = b_pool.tile([P, MT], F32, tag="bias")
            nc.sync.dma_start(bias_sb[:], b_d[:, :])

            # Resident x, 8 chunks of 4 k-tiles (1 MB each), in k order on
            # one HWDGE queue (FIFO) so chunk g lands just ahead of the PE's
            # k=4g matmuls. First chunk is split so the k=0 matmul can start
            # ~2us earlier.
            xt = []
            for g in range(NXG):
                th = x_pool.tile([P, XG, T], FP16, tag=f"xh_{g}", name=f"xh_{g}")
                if g == 0:
                    nc.sync.dma_start(th[:, 0:2, :], x_d[:, 0 : 2 * T])
                    nc.sync.dma_start(th[:, 2:4, :], x_d[:, 2 * T : 4 * T])
                else:
                    nc.sync.dma_start(th[:], x_d[:, g * XG * T : (g + 1) * XG * T])
                xt.append(th)

            def x_sl(k, n):
                return xt[k // XG][:, k % XG, n * TB : (n + 1) * TB]

            # mm_gates[(mb, k)]: a matmul to gate later W-block DMAs on, so
            # the W prefetch does not steal HBM bandwidth from the x stream
            # during the first block (x must fully land before mb=0 ends).
            gate_for_wdma = {1: (0, 24), 2: (1, 8)}
            mm_gates = {}
            for mb in range(MB):
                wsb = w_pool.tile([P, KT, M2 * P], FP8, tag="wsb",
                                  name=f"wsb_{mb}")
                if mb == 0:
                    # 4 pieces so the k=0 LDWEIGHTS only waits on a 256 KB
                    # transfer.
                    for q in range(4):
                        nc.gpsimd.dma_start(
                            wsb[:, q * 8 : (q + 1) * 8, :],
                            w_d[0:P, q * 8 * M2 * P : (q + 1) * 8 * M2 * P],
                        )
                else:
                    dma = nc.gpsimd.dma_start(wsb[:], w_d[mb * P : (mb + 1) * P, :])
                    if mb in gate_for_wdma:
                        add_dep_helper(dma.ins, mm_gates[gate_for_wdma[mb]],
                                       reason="delay W prefetch past x stream")

                psums = {
                    (mi, n): psum_pool.tile([P, TB], F32, tag="psum",
                                            name=f"ps_{mb}_{n}_{mi}")
                    for mi in range(M2)
                    for n in range(NB)
                }
                for k in range(KT):
                    for mi in range(M2):
                        msl = slice(mi * P, (mi + 1) * P)
                        for n in range(NB):
                            mm = nc.tensor.matmul(
                                psums[(mi, n)][:],
                                wsb[:, k, msl],
                                x_sl(k, n),
                                start=(k == 0),
                                stop=(k == KT - 1),
                            )
                            if mi == 0 and n == 0:
                                mm_gates[(mb, k)] = mm.ins
                for mi in range(M2):
                    m = mb * M2 + mi
                    for n in range(NB):
                        osb = out_pool.tile([P, TB], FP8, tag="osb",
                                            name=f"osb_{mb}_{n}_{mi}")
                        nc.scalar.activation(
                            osb[:], psums[(mi, n)][:], SIGN,
                            bias=bias_sb[:, m : m + 1],
                        )
                        nc.sync.dma_start(
                            out_d[m * P : (m + 1) * P, n * TB : (n + 1) * TB],
                            osb[:],
                        )
    nc.compile()
    _nc_cache = nc
    return nc


def prep_in_maps(x, weight, bias):
    """Host-side layout prep: fp16 cast of x, sign(W)->fp8, packing."""
    x = np.asarray(x, dtype=np.float32)
    weight = np.asarray(weight, dtype=np.float32)
    bias = np.asarray(bias, dtype=np.float32)

    x16 = x.astype(np.float16)
    # w8[p, k, mi, c] = sign(W)[mb*256 + mi*128 + c, k*128 + p], per block mb
    S = np.sign(weight).astype(np.float32)
    w8 = (
        S.reshape(MB, M2, P, KT, P)
        .transpose(0, 4, 3, 1, 2)  # [mb, p, k, mi, c]
        .reshape(MB * P, KT * M2 * P)
    ).astype(E4M3)
    w8 = np.ascontiguousarray(w8)
    bias_pk = np.ascontiguousarray(bias.reshape(MT, P).T)

    in_maps = []
    for c in range(N_CORES):
        xc = x16[c * T : (c + 1) * T]  # [T, D_IN]
        # xp[p, k, t] = x16[c*T + t, k*128 + p]
        xp = np.ascontiguousarray(
            xc.reshape(T, KT, P).transpose(2, 1, 0).reshape(P, KT * T)
        )
        in_maps.append({"x16": xp, "w8": w8, "bias_pk": bias_pk})
    return in_maps


def run(x, weight, bias, **spmd_kwargs):
    """Run on the 8 cores; returns (full_output, BassKernelResults)."""
    nc = build()
    in_maps = prep_in_maps(x, weight, bias)
    res = run_bass_kernel_spmd(nc, in_maps, core_ids=list(range(N_CORES)), **spmd_kwargs)
    out = np.empty((N_TOK, D_OUT), dtype=np.float32)
    for c in range(N_CORES):
        out[c * T : (c + 1) * T, :] = res.results[c]["out8_t"].astype(np.float32).T
    return out, res


def kernel(x, weight, bias):
    out, _ = run(x, weight, bias)
    return out


# revision 7
# speedup vs baseline: 1.8060x; 1.0038x over previous
"""Trainium2 Bass kernel for nn_BinaryLinear: out = sign(x @ sign(W).T + bias).

Strategy
--------
Data-parallel over the 8192-token dim: each of the 8 cores gets 1024 tokens
and the full weight matrix.

On-chip compute (per core) is the NT GEMM z.T = sign(W) @ x.T on the
TensorEngine with the contraction (in_features) on the partition dim:

  psum[outf, tok] = sum_k w_s[k, outf] * x16[k, tok]

Precision: x is rounded to fp16 on the host. Host-side simulation of the
exact quantization error (acc_sim.py) shows 2195/33.5M sign flips vs the
fp32 reference => rel_err 0.0162 < 2e-2 gate (the sim matched HW flip
counts exactly for the previous fp16+fp8 scheme: 97 predicted, 97
measured). PE accumulation is fp32 and adds nothing measurable.

sign(W) is computed on the host and shipped as fp8e4 (+-1 exact), already
packed in the SBUF layout the PE wants (per-partition-contiguous), so every
DMA is a full-line contiguous transfer and no on-chip conversion is needed.
The matmul runs with fp8 stationary x fp16 moving at 1 col/cycle.

The epilogue fuses bias-add + sign + PSUM->SBUF in one ScalarE activation
(bias is per-partition in the z.T layout), writing fp8 (+-1 exact), which
quarters the output DMA. Host converts back to fp32 and untransposes.

DMA queues: x + bias on sync (HWDGE), W blocks on gpsimd (SWDGE),
outputs on scalar (HWDGE) so the three streams don't serialize behind
each other in one FIFO.
"""

import numpy as np

import concourse.tile as tile
import concourse.mybir as mybir
from concourse import bacc
from concourse.bass_utils import run_bass_kernel_spmd
from concourse.tile_rust import add_dep_helper

N_CORES = 8
N_TOK = 8192
D_IN = 4096
D_OUT = 4096
P = 128
T = N_TOK // N_CORES  # 1024 tokens per core
KT = D_IN // P  # 32 contraction tiles
MT = D_OUT // P  # 32 out-feature tiles
M2 = 2  # m-tiles per W block (256 outf cols)
MB = MT // M2  # 16 W blocks
TB = 512  # token block (one PSUM bank of fp32)
NB = T // TB  # 2 token blocks per core
XG = 2  # k-tiles per x DMA chunk
NXG = KT // XG  # 16 x DMA chunks

F32 = mybir.dt.float32
FP16 = mybir.dt.float16
FP8 = mybir.dt.float8e4
SIGN = mybir.ActivationFunctionType.Sign
E4M3 = mybir.dt.np(FP8)

_nc_cache = None


def build():
    """Build + compile the per-core Bass/Tile module (SPMD: same on all cores)."""
    global _nc_cache
    if _nc_cache is not None:
        return _nc_cache
    nc = bacc.Bacc("TRN2", target_bir_lowering=False, debug=False, num_devices=N_CORES)
    x_d = nc.dram_tensor("x16", [P, KT * T], FP16, kind="ExternalInput").ap()
    w_d = nc.dram_tensor("w8", [MB * P, KT * M2 * P], FP8, kind="ExternalInput").ap()
    b_d = nc.dram_tensor("bias_pk", [P, MT], F32, kind="ExternalInput").ap()
    out_d = nc.dram_tensor("out8_t", [D_OUT, T], FP8, kind="ExternalOutput").ap()

    with tile.TileContext(nc) as tc:
        with (
            tc.tile_pool(name="x", bufs=1) as x_pool,
            tc.tile_pool(name="wsb", bufs=3) as w_pool,
            tc.tile_pool(name="bias", bufs=1) as b_pool,
            tc.tile_pool(name="out", bufs=6) as out_pool,
            tc.tile_pool(name="warm", bufs=1) as warm_pool,
            tc.tile_pool(name="psum", bufs=8, space="PSUM") as psum_pool,
        ):
            # PE warmup: the HAM clock gate holds the PE at 1.2 GHz until it
            # has been busy ~3.4us. Run dummy matmuls on a zeroed tile while
            # the first x/W DMAs are in flight so the real matmuls start at
            # 2.4 GHz.
            warm = warm_pool.tile([P, TB], FP16, tag="warm")
            nc.gpsimd.memset(warm[:], 0.0)
            wps = psum_pool.tile([P, TB], F32, tag="psum", name="warm_ps")
            for i in range(14):
                nc.tensor.matmul(wps[:], warm[:, :P], warm[:], start=True,
                                 stop=True)

            # bias, outf-partition-major: bias_sb[p, mo] = bias[mo*128 + p]
            bias_sb = b_pool.tile([P, MT], F32, tag="bias")
            nc.sync.dma_start(bias_sb[:], b_d[:, :])

            # Resident x, 8 chunks of 4 k-tiles (1 MB each), in k order on
            # one HWDGE queue (FIFO) so chunk g lands just ahead of the PE's
            # k=4g matmuls. First chunk is split so the k=0 matmul can start
            # ~2us earlier.
            xt = []
            for g in range(NXG):
                th = x_pool.tile([P, XG, T], FP16, tag=f"xh_{g}", name=f"xh_{g}")
                nc.sync.dma_start(th[:], x_d[:, g * XG * T : (g + 1) * XG * T])
                xt.append(th)

            def x_sl(k, n):
                return xt[k // XG][:, k % XG, n * TB : (n + 1) * TB]

            # mm_gates[(mb, k)]: a matmul to gate later W-block DMAs on, so
            # the W prefetch does not steal HBM bandwidth from the x stream
            # during the first block (x must fully land before mb=0 ends).
            gate_for_wdma = {1: (0, 24), 2: (1, 8)}
            mm_gates = {}
            for mb in range(MB):
                wsb = w_pool.tile([P, KT, M2 * P], FP8, tag="wsb",
                                  name=f"wsb_{mb}")
                if mb == 0:
                    # 4 pieces so the k=0 LDWEIGHTS only waits on a 256 KB
                    # transfer.
                    for q in range(4):
                        nc.gpsimd.dma_start(
                            wsb[:, q * 8 : (q + 1) * 8, :],
                            w_d[0:P, q * 8 * M2 * P : (q + 1) * 8 * M2 * P],
                        )
                else:
                    dma = nc.gpsimd.dma_start(wsb[:], w_d[mb * P : (mb + 1) * P, :])
                    if mb in gate_for_wdma:
                        add_dep_helper(dma.ins, mm_gates[gate_for_wdma[mb]],
                                       reason="delay W prefetch past x stream")

                psums = {
                    (mi, n): psum_pool.tile([P, TB], F32, tag="psum",
                                            name=f"ps_{mb}_{n}_{mi}")
                    for mi in range(M2)
                    for n in range(NB)
                }
                for k in range(KT):
                    for mi in range(M2):
                        msl = slice(mi * P, (mi + 1) * P)
                        for n in range(NB):
                            mm = nc.tensor.matmul(
                                psums[(mi, n)][:],
                                wsb[:, k, msl],
                                x_sl(k, n),
                                start=(k == 0),
                                stop=(k == KT - 1),
                            )
                            if mi == 0 and n == 0:
                                mm_gates[(mb, k)] = mm.ins
                for mi in range(M2):
                    m = mb * M2 + mi
                    for n in range(NB):
                        osb = out_pool.tile([P, TB], FP8, tag="osb",
                                            name=f"osb_{mb}_{n}_{mi}")
                        nc.scalar.activation(
                            osb[:], psums[(mi, n)][:], SIGN,
                            bias=bias_sb[:, m : m + 1],
                        )
                        nc.sync.dma_start(
                            out_d[m * P : (m + 1) * P, n * TB : (n + 1) * TB],
                            osb[:],
                        )
    nc.compile()
    _nc_cache = nc
    return nc


def prep_in_maps(x, weight, bias):
    """Host-side layout prep: fp16 cast of x, sign(W)->fp8, packing."""
    x = np.asarray(x, dtype=np.float32)
    weight = np.asarray(weight, dtype=np.float32)
    bias = np.asarray(bias, dtype=np.float32)

    x16 = x.astype(np.float16)
    # w8[p, k, mi, c] = sign(W)[mb*256 + mi*128 + c, k*128 + p], per block mb
    S = np.sign(weight).astype(np.float32)
    w8 = (
        S.reshape(MB, M2, P, KT, P)
        .transpose(0, 4, 3, 1, 2)  # [mb, p, k, mi, c]
        .reshape(MB * P, KT * M2 * P)
    ).astype(E4M3)
    w8 = np.ascontiguousarray(w8)
    bias_pk = np.ascontiguousarray(bias.reshape(MT, P).T)

    in_maps = []
    for c in range(N_CORES):
        xc = x16[c * T : (c + 1) * T]  # [T, D_IN]
        # xp[p, k, t] = x16[c*T + t, k*128 + p]
        xp = np.ascontiguousarray(
            xc.reshape(T, KT, P).transpose(2, 1, 0).reshape(P, KT * T)
        )
        in_maps.append({"x16": xp, "w8": w8, "bias_pk": bias_pk})
    return in_maps


def run(x, weight, bias, **spmd_kwargs):
    """Run on the 8 cores; returns (full_output, BassKernelResults)."""
    nc = build()
    in_maps = prep_in_maps(x, weight, bias)
    res = run_bass_kernel_spmd(nc, in_maps, core_ids=list(range(N_CORES)), **spmd_kwargs)
    out = np.empty((N_TOK, D_OUT), dtype=np.float32)
    for c in range(N_CORES):
        out[c * T : (c + 1) * T, :] = res.results[c]["out8_t"].astype(np.float32).T
    return out, res


def kernel(x, weight, bias):
    out, _ = run(x, weight, bias)
    return out
